# revision 16
# baseline (speedup 1.0000x reference)
"""Multi-head attention (B=2, S=4096, E=512, H=8) on 8 trn2 NeuronCores.

Sharding: 16 (batch, head) pairs -> 2 heads per core (core c: batch c//4,
heads 2*(c%4), 2*(c%4)+1). Each core computes q/k/v projections for its two
heads, full (unscaled-softmax) attention, and a partial output projection
through its rows of Wo. Host sums the 4 partial outputs per batch and adds
the bias terms (bo + bv @ Wo, exact since softmax rows sum to 1).

Device layout notes:
  - activations are fed pre-transposed ([E, S]) so every matmul contracts
    along partitions with fully contiguous DMA.
  - scores are computed transposed ([s_k, s_q] tiles) so exp(scores) tiles
    can be used directly as the stationary operand of the attention matmul.
  - softmax denominators come for free from a ones-column appended to v
    (output column 64 of the attention matmul).
"""

import numpy as np

import concourse.bass as bass
import concourse.mybir as mybir
from concourse.tile import TileContext
from concourse.bass_utils import run_bass_kernel_spmd

B, S, E, H = 2, 4096, 512, 8
DK = E // H  # 64
N_CORES = 8
F32 = mybir.dt.float32
BF16 = mybir.dt.bfloat16
AF = mybir.ActivationFunctionType
ADD = mybir.AluOpType.add
MULT = mybir.AluOpType.mult

# ---------------------------------------------------------------------------
# Workaround for walrus "Too many sync wait commands" on the TileContext
# final drain: emit one single-wait SP nop per pending semaphore before the
# drain, and emit the drain itself with no waits.
# ---------------------------------------------------------------------------
import bass_rust

_patched = False


def _split_drain_and_barrier(self, tick_clock, wait_clock):
    gc = tick_clock.global_clock
    counts = eval(repr(gc).replace("VectorClock", ""))
    for proc, cnt in enumerate(counts):
        if cnt <= 0:
            continue
        single = [0] * len(counts)
        single[proc] = cnt
        nop = self.nc.sync.nop(nofuse=True, hint="drain_split")
        wait_clock.add_sem_waits(
            nop.ins, bass_rust.ScopedClock({None: bass_rust.VectorClock(single)})
        )
    self.nc.sync.drain()
    self.nc.all_engine_barrier()
    assert self.sems is not None
    popped = self.nc._tile_sem_poison_stack.pop()
    assert popped is self._sem_poison
    self.nc.clear_and_free_semaphores(list(self.sems.allocated().values()))
    self.nc.all_engine_barrier()


_orig_saa = TileContext.schedule_and_allocate


def _saa_capture(self, *a, **k):
    r = _orig_saa(self, *a, **k)
    try:
        self.predicted_ns = r[1].time if r and r[1] is not None else None
    except Exception:
        self.predicted_ns = None
    return r


def _apply_patch():
    global _patched
    if not _patched:
        TileContext._drain_and_barrier = _split_drain_and_barrier
        TileContext.schedule_and_allocate = _saa_capture
        _patched = True


def _split_multiwait_json(raw: bytes) -> bytes:
    """The walrus build in this container accepts at most ONE sync wait per
    instruction. Hoist extra waits onto single-wait NoOps spliced in just
    before the instruction on the same engine stream (engine streams follow
    block order, so the nops complete before the instruction issues)."""
    import orjson

    j = orjson.loads(raw)
    n_split = 0
    for f in j["functions"]:
        for bb in f["blocks"]:
            out = []
            for inst in bb["instructions"]:
                si = inst.get("sync_info") or {}
                ow = si.get("on_wait") or []
                if len(ow) > 1:
                    for i, w in enumerate(ow[:-1]):
                        out.append(
                            {
                                "name": f"{inst['name']}-wsplit{i}",
                                "opcode": "NoOp",
                                "engine": inst["engine"],
                                "ins": [],
                                "outs": [],
                                "sync_info": {"on_wait": [w], "on_update": []},
                            }
                        )
                        n_split += 1
                    si["on_wait"] = [ow[-1]]
                out.append(inst)
            bb["instructions"] = out
    return orjson.dumps(j)


def _patch_to_json(nc):
    orig = nc.to_json_bytes

    def wrapped(*a, **k):
        return _split_multiwait_json(orig(*a, **k))

    nc.to_json_bytes = wrapped
    return nc


# ---------------------------------------------------------------------------
# Kernel builder (per-core program; SPMD over 8 cores with different data)
# ---------------------------------------------------------------------------


def build(s=S, reps=1):
    """Build the per-core Bass program for sequence length s. reps>1 wraps
    the whole body in an on-device For_i loop (used only for timing)."""
    import contextlib
    from concourse.masks import make_identity

    assert s % 512 == 0
    SC = s // 512  # 512-wide s chunks
    KC = s // 128  # 128-wide kv chunks
    nc = bass.Bass(target_bir_lowering=False, trn_type="TRN2")

    F32R = mybir.dt.float32r
    xq = nc.dram_tensor("xq", [E, s], F32R, kind="ExternalInput")
    xk = nc.dram_tensor("xk", [E, s], F32R, kind="ExternalInput")
    xv = nc.dram_tensor("xv", [E, s], F32, kind="ExternalInput")
    wq = nc.dram_tensor("wq", [E, 128], F32R, kind="ExternalInput")
    wk = nc.dram_tensor("wk", [E, 128], F32R, kind="ExternalInput")
    wv = nc.dram_tensor("wv", [E, 128], F32, kind="ExternalInput")
    wo = nc.dram_tensor("wo", [128, E], F32, kind="ExternalInput")
    bq2 = nc.dram_tensor("bq2", [128, 1], F32, kind="ExternalInput")
    bk2 = nc.dram_tensor("bk2", [128, 1], F32, kind="ExternalInput")
    outT = nc.dram_tensor("outT", [E, s], F32, kind="ExternalOutput")

    with TileContext(nc) as tc:
        with contextlib.ExitStack() as _stack:
            constp = _stack.enter_context(tc.tile_pool(name="const", bufs=1))
            wstage = _stack.enter_context(tc.tile_pool(name="wstage", bufs=2))
            if reps > 1:
                _stack.enter_context(tc.For_i(0, reps, 1))
            # --- weights: wq/wk stay f32r (full-rate fp32 matmul);
            #     wv is cast to bf16 ---
            wbs = {}
            for name, dram in (("wq", wq), ("wk", wk)):
                rt = constp.tile([128, 512], F32R, tag=f"{name}_r", name=f"{name}_r")
                for ec in range(4):
                    nc.sync.dma_start(
                        rt[:, ec * 128 : (ec + 1) * 128],
                        dram[ec * 128 : (ec + 1) * 128, :],
                    )
                wbs[name] = rt
            f32t = wstage.tile([128, 512], F32, tag="wf")
            for ec in range(4):
                nc.sync.dma_start(
                    f32t[:, ec * 128 : (ec + 1) * 128],
                    wv[ec * 128 : (ec + 1) * 128, :],
                )
            wv_b = constp.tile([128, 512], BF16, tag="wv_b")
            nc.vector.tensor_copy(wv_b[:], f32t[:])
            wbs["wv"] = wv_b
            wo_f = wstage.tile([128, 512], F32, tag="wf")
            nc.sync.dma_start(wo_f[:], wo[:, :])
            wo_b = constp.tile([128, 512], BF16, tag="wo_b")
            nc.vector.tensor_copy(wo_b[:], wo_f[:])

            bq_t = constp.tile([128, 1], F32, tag="bq")
            nc.sync.dma_start(bq_t[:], bq2[:, :])
            bk_t = constp.tile([128, 1], F32, tag="bk")
            nc.sync.dma_start(bk_t[:], bk2[:, :])

            ident = constp.tile([128, 128], BF16, tag="ident")
            make_identity(nc, ident[:])

            # persistent activations
            qT = constp.tile([128, s], F32R, tag="qT")
            kT = constp.tile([128, s], F32R, tag="kT")
            attnT = constp.tile([128, s], BF16, tag="attnT")
            v2 = constp.tile([128, KC * 130], BF16, tag="v2")
            nc.gpsimd.memset(v2[:], 1.0)

            # ---------------- stage A: projections ----------------
            with (
                tc.tile_pool(name="xin", bufs=6) as xinp,
                tc.tile_pool(name="xbp", bufs=6) as xbp,
                tc.tile_pool(name="projp", bufs=3, space="PSUM") as projp,
                tc.tile_pool(name="vprojp", bufs=2, space="PSUM") as vprojp,
            ):
                for sc in range(SC):
                    for dram, wname, btile, dst in (
                        (xq, "wq", bq_t, qT),
                        (xk, "wk", bk_t, kT),
                    ):
                        ps = projp.tile([128, 512], F32, tag="proj")
                        for ec in range(4):
                            xt = xinp.tile([128, 512], F32R, tag="xin")
                            nc.sync.dma_start(
                                xt[:],
                                dram[
                                    ec * 128 : (ec + 1) * 128,
                                    sc * 512 : (sc + 1) * 512,
                                ],
                            )
                            nc.tensor.matmul(
                                ps[:],
                                wbs[wname][:, ec * 128 : (ec + 1) * 128],
                                xt[:],
                                start=(ec == 0),
                                stop=(ec == 3),
                            )
                        nc.vector.tensor_scalar(
                            out=dst[:, sc * 512 : (sc + 1) * 512],
                            in0=ps[:],
                            scalar1=btile[:],
                            scalar2=None,
                            op0=ADD,
                        )
                    # v projection: natural [s, dk2] layout into v2
                    xvbs = []
                    for ec in range(4):
                        xt = xinp.tile([128, 512], F32, tag="xin")
                        nc.sync.dma_start(
                            xt[:],
                            xv[ec * 128 : (ec + 1) * 128, sc * 512 : (sc + 1) * 512],
                        )
                        xb = xbp.tile([128, 512], BF16, tag="xb")
                        nc.vector.tensor_copy(xb[:], xt[:])
                        xvbs.append(xb)
                    for mc in range(4):
                        psv = vprojp.tile([128, 128], F32, tag="vproj")
                        for ec in range(4):
                            nc.tensor.matmul(
                                psv[:],
                                xvbs[ec][:, mc * 128 : (mc + 1) * 128],
                                wbs["wv"][:, ec * 128 : (ec + 1) * 128],
                                start=(ec == 0),
                                stop=(ec == 3),
                            )
                        t = sc * 4 + mc
                        nc.vector.tensor_copy(
                            v2[:, t * 130 : t * 130 + 64], psv[:, 0:64]
                        )
                        nc.vector.tensor_copy(
                            v2[:, t * 130 + 65 : t * 130 + 129], psv[:, 64:128]
                        )

            # ---------------- stage B: attention ----------------
            with (
                tc.tile_pool(name="wt", bufs=KC + 2) as wtp,
                tc.tile_pool(name="misc", bufs=10) as misc,
                tc.tile_pool(name="scoresp", bufs=2, space="PSUM") as scoresp,
                tc.tile_pool(name="attnp", bufs=2, space="PSUM") as attnp,
                tc.tile_pool(name="tposep", bufs=1, space="PSUM") as tposep,
                tc.tile_pool(name="outp", bufs=1, space="PSUM") as outp,
            ):
                for sq in range(SC):
                    # exp'd score tiles, indexed by (kc, h); both heads of a
                    # kv chunk share one 2-bank psum region and one exp.
                    wts = {}
                    for kc in range(KC):
                        ps = scoresp.tile([128, 1024], F32, tag="sc")
                        for h in (0, 1):
                            nc.tensor.matmul(
                                ps[:, h * 512 : (h + 1) * 512],
                                kT[h * 64 : (h + 1) * 64, kc * 128 : (kc + 1) * 128],
                                qT[h * 64 : (h + 1) * 64, sq * 512 : (sq + 1) * 512],
                                start=True,
                                stop=True,
                            )
                        wt = wtp.tile([128, 1024], BF16, tag="wt")
                        nc.scalar.activation(wt[:], ps[:], AF.Exp)
                        for h in (0, 1):
                            wts[(kc, h)] = wt[:, h * 512 : (h + 1) * 512]
                    pairs = []
                    for m in range(4):
                        pairs.append(misc.tile([128, 128], BF16, tag="pair", name=f"pair_{sq}_{m}"))
                    for h in (0, 1):
                        for m in range(4):
                            aps = attnp.tile([128, 65], F32, tag="at")
                            for kc in range(KC):
                                nc.tensor.matmul(
                                    aps[:],
                                    wts[(kc, h)][:, m * 128 : (m + 1) * 128],
                                    v2[:, kc * 130 + h * 65 : kc * 130 + (h + 1) * 65],
                                    start=(kc == 0),
                                    stop=(kc == KC - 1),
                                )
                            rcp = misc.tile([128, 1], F32, tag="rcp")
                            nc.vector.reciprocal(rcp[:], aps[:, 64:65])
                            nc.vector.tensor_scalar(
                                out=pairs[m][:, h * 64 : (h + 1) * 64],
                                in0=aps[:, 0:64],
                                scalar1=rcp[:],
                                scalar2=None,
                                op0=MULT,
                            )
                    for m in range(4):
                        tp = tposep.tile([128, 128], BF16, tag="tp")
                        nc.tensor.transpose(tp[:], pairs[m][:], ident[:])
                        nc.vector.tensor_copy(
                            attnT[:, sq * 512 + m * 128 : sq * 512 + (m + 1) * 128],
                            tp[:],
                        )
                    for oc in range(4):
                        po = outp.tile([128, 512], F32, tag="po")
                        nc.tensor.matmul(
                            po[:],
                            wo_b[:, oc * 128 : (oc + 1) * 128],
                            attnT[:, sq * 512 : (sq + 1) * 512],
                            start=True,
                            stop=True,
                        )
                        ost = misc.tile([128, 512], F32, tag="ost", name=f"ost_{sq}_{oc}")
                        nc.vector.tensor_copy(ost[:], po[:])
                        nc.sync.dma_start(
                            outT[oc * 128 : (oc + 1) * 128, sq * 512 : (sq + 1) * 512],
                            ost[:],
                        )
    nc._predicted_ns = getattr(tc, "predicted_ns", None)
    return _patch_to_json(nc)


# ---------------------------------------------------------------------------
# Host-side sharding / gathering
# ---------------------------------------------------------------------------


def make_in_maps(query, key_in, value, Wq, bq, Wk, bk, Wv, bv, Wo, bo, s=S):
    in_maps = []
    for c in range(N_CORES):
        b = c // 4
        hs = (c % 4) * 2 * DK  # column offset of this core's two heads
        in_maps.append(
            {
                "xq": np.ascontiguousarray(query[b, :s].T),
                "xk": np.ascontiguousarray(key_in[b, :s].T),
                "xv": np.ascontiguousarray(value[b, :s].T),
                "wq": np.ascontiguousarray(Wq[:, hs : hs + 128]),
                "wk": np.ascontiguousarray(Wk[:, hs : hs + 128]),
                "wv": np.ascontiguousarray(Wv[:, hs : hs + 128]),
                "wo": np.ascontiguousarray(Wo[hs : hs + 128, :]),
                "bq2": np.ascontiguousarray(bq[hs : hs + 128, None]),
                "bk2": np.ascontiguousarray(bk[hs : hs + 128, None]),
            }
        )
    return in_maps


def assemble(results, bv, Wo, bo, s=S):
    out = np.zeros((B, s, E), np.float32)
    for c in range(N_CORES):
        out[c // 4] += results[c]["outT"].T
    out += (bo + bv @ Wo)[None, None, :]
    return out


_nc_cache = {}


def kernel(query, key_in, value, Wq, bq, Wk, bk, Wv, bv, Wo, bo):
    _apply_patch()
    query = np.asarray(query, np.float32)
    key_in = np.asarray(key_in, np.float32)
    value = np.asarray(value, np.float32)
    Wq, bq = np.asarray(Wq, np.float32), np.asarray(bq, np.float32)
    Wk, bk = np.asarray(Wk, np.float32), np.asarray(bk, np.float32)
    Wv, bv = np.asarray(Wv, np.float32), np.asarray(bv, np.float32)
    Wo, bo = np.asarray(Wo, np.float32), np.asarray(bo, np.float32)

    if S not in _nc_cache:
        _nc_cache[S] = build(S)
    nc = _nc_cache[S]
    in_maps = make_in_maps(query, key_in, value, Wq, bq, Wk, bk, Wv, bv, Wo, bo)
    res = run_bass_kernel_spmd(nc, in_maps, core_ids=list(range(N_CORES)))
    return assemble(res.results, bv, Wo, bo)
